# revision 30
# baseline (speedup 1.0000x reference)
"""Distributed masked multi-head self-attention for Trainium2 (8 NeuronCores).

Problem: x:[2,2048,1024], mask:[2,2048], Wq:[1024,1024], Wkv:[1024,2048],
Wo:[1024,1024]  ->  out:[2,2048,1024]  (fp32)

Strategy (single SPMD launch, one NEFF on 8 cores):
  Phase 1 (head parallel): core c owns heads {2c, 2c+1} (128 contiguous
    columns of q/k/v).  Each core reads full x (transposed on host to
    [b, dim, n]) and computes qT/kT = [128, n] (head-dim major) and
    v = [n, 128] for its heads, then masked softmax attention entirely
    in "scores transposed" [key, query] layout, processed in 512-query
    blocks (QW=512) so PSUM banks allow double-buffered score tiles:
      simT = kT_h slice as lhsT, qT as rhs -> [128 keys, 512q] in PSUM
      P    = exp(scale*simT + mask_bias_per_key_partition)   (ScalarE)
      outT[64h:, q] += v_h^T @ P  (ones column in v^T also accumulates
      the softmax denominator into psum row 64)
  The per-(batch, half) chunk of normalized att^T [128 cols, 1024 rows]
  is AllToAll'd as soon as it is ready (4 chunks total, overlapped with
  later attention blocks): dst core j receives rows j*128..j*128+128 of
  the chunk from every src core -> [1024 cols, 128 rows].
  Phase 2 (row parallel, per chunk): out_rows = att_rows @ Wo with full
  Wo; core c writes 128 rows per chunk (4*128 = 512 rows total, in chunk
  order).  Host reassembles the row permutation.

  The Tile list scheduler overlaps phases: batch-1 projections fill PE
  gaps during batch-0 attention (which is ScalarE-exp-bound), chunk
  collectives and out-projections overlap later attention blocks.

Precision tiers (measured max rel err ~4e-3 vs fp64):
  - x, Wq/Wk/Wv:       bf16   (halves the dominant x-broadcast DMA)
  - q/k, scores, proj: float32r (fp32 storage, full PE rate)
  - P=exp(s), v, att, Wo: bf16 (halves the AllToAll payload)
  - PSUM accumulation, softmax denominators, output: fp32
ATT_MM_MODE=f32 switches the f32r tier to exact fp32 (4x slower PE).
"""

import os
import sys

import numpy as np

for _p in ("/opt/trn_rl_repo",):
    if _p not in sys.path and os.path.isdir(_p):
        sys.path.append(_p)

import concourse.bass as bass
from concourse import bacc
import concourse.mybir as mybir
import concourse.tile as tile
from concourse.masks import make_identity
from contextlib import ExitStack

# ----- problem constants (hardcoded; kernel.py must be self-contained) -----
B, N, DIM, H, DH = 2, 2048, 1024, 16, 64
DI = H * DH                       # 1024
NCORES = 8
HPC = H // NCORES                 # 2 heads per core
CW = HPC * DH                     # 128 att columns per core
RPC = B * N // NCORES             # 512 output rows per core
SCALE = DH ** -0.5
MASK_NEG = -30000.0               # exp(scale*s + MASK_NEG) == 0 in fp32

P = 128                           # partitions
KT = DIM // P                     # 8 contraction tiles for projections
NKT = N // P                      # 16 key tiles
QW = 512                          # query block width
NJH = N // QW                     # 4 query blocks per batch
CPB = int(os.environ.get("ATT_CPB", "2"))   # a2a chunks per batch (1 or 2)
NSUB = NJH // CPB                 # query blocks per chunk
NCH = CPB * B                     # total a2a chunks
CR = NSUB * QW // NCORES          # rows per core per chunk
FP32 = mybir.dt.float32
F32R = mybir.dt.float32r
BF16 = mybir.dt.bfloat16

MM_MODE = os.environ.get("ATT_MM_MODE", "f32r")  # "f32r" | "f32"
# timing-only ablations (wrong numerics; never set when grading)
ABLATE_SB = os.environ.get("ATT_ABLATE_SB") == "1"    # drop 2nd score matmul
ABLATE_PV = os.environ.get("ATT_ABLATE_PV") == "1"    # drop 2nd pv matmul
ABLATE_COLL = os.environ.get("ATT_ABLATE_COLL") == "1"  # drop collectives
# float32r: fp32 storage, full PE rate for free dim >= 256.  The BIR
# verifier requires every producer of an f32r-matmul operand to emit
# f32r, so those tensors are declared f32r end-to-end.
MDT = F32R if MM_MODE == "f32r" else FP32
PDT = BF16   # post-softmax path: P, v, Wo
# a2a payload (normalized att weights in [0,1]); fp8e4m3 halves the
# collective bytes and the quantization error averages out over the
# 1024-wide out-proj contraction
ADT = mybir.dt.float8e4 if os.environ.get("ATT_A2A_F8", "0") == "1" else BF16
# att weights are ~1/n_valid_keys (~1e-3) — below e4m3's normal range; carry
# a x256 scale through the a2a (folded into the softmax denominator) and
# divide it back out after the out-projection
A2A_S = 256.0 if ADT == mybir.dt.float8e4 else 1.0


def build_program(reps=1):
    nc = bacc.Bacc(None, target_bir_lowering=False, num_devices=NCORES)

    xt = nc.dram_tensor("xt", [B, DIM, N], BF16, kind="ExternalInput")
    wq = nc.dram_tensor("wq", [DIM, CW], BF16, kind="ExternalInput")
    wk = nc.dram_tensor("wk", [DIM, CW], BF16, kind="ExternalInput")
    wv = nc.dram_tensor("wv", [DIM, CW], BF16, kind="ExternalInput")
    wo = nc.dram_tensor("wo", [DI, DIM], PDT, kind="ExternalInput")
    mb = nc.dram_tensor("mb", [B, P, NKT], FP32, kind="ExternalInput")
    out = nc.dram_tensor("out", [RPC, DIM], FP32, kind="ExternalOutput")

    with tile.TileContext(nc) as tc, ExitStack() as ctx, \
            nc.allow_low_precision(reason="f32r matmul pipeline; psum accum stays fp32"):
        const = ctx.enter_context(tc.tile_pool(name="const", bufs=1))
        wts = ctx.enter_context(tc.tile_pool(name="wts", bufs=1))
        xtp = ctx.enter_context(tc.tile_pool(name="xtp", bufs=16))
        qkp = ctx.enter_context(tc.tile_pool(name="qkp", bufs=2))
        vtp = ctx.enter_context(tc.tile_pool(name="vtp", bufs=2))
        vsp = ctx.enter_context(tc.tile_pool(name="vsp", bufs=2))
        pp = ctx.enter_context(tc.tile_pool(name="pp", bufs=8))
        sml = ctx.enter_context(tc.tile_pool(name="sml", bufs=4))
        atc = ctx.enter_context(tc.tile_pool(name="atc", bufs=2))
        a2s = ctx.enter_context(tc.tile_pool(name="a2s", bufs=2))
        osp = ctx.enter_context(tc.tile_pool(name="osp", bufs=2))
        ps = ctx.enter_context(tc.tile_pool(name="ps", bufs=1, space="PSUM"))
        dram = ctx.enter_context(tc.tile_pool(name="dram", bufs=2, space="DRAM"))

        # ---- constants / weights (resident across reps) ----
        identity = const.tile([P, P], FP32, tag="ident")
        make_identity(nc, identity)
        ident_bf = const.tile([P, P], BF16, tag="identbf")
        nc.vector.tensor_copy(ident_bf[:], identity[:])
        ones_f32 = const.tile([P, 64], FP32, tag="ones_f32")
        nc.vector.memset(ones_f32[:], 1.0)
        invs_f32 = const.tile([P, 64], FP32, tag="invs_f32")
        nc.vector.memset(invs_f32[:], 1.0 / A2A_S)
        ones_col = const.tile([1, 64], MDT, tag="ones_col")   # bcast lhsT
        nc.vector.tensor_copy(ones_col[:], ones_f32[0:1, :])
        mb_sb = const.tile([P, B, NKT], FP32, tag="mb")
        nc.sync.dma_start(out=mb_sb[:], in_=mb.rearrange("b p t -> p b t"))

        wq_sb = wts.tile([P, KT, CW], BF16, tag="wq")
        wk_sb = wts.tile([P, KT, CW], BF16, tag="wk")
        wv_sb = wts.tile([P, KT, CW], BF16, tag="wv")
        for w_sb, w in ((wk_sb, wk), (wv_sb, wv), (wq_sb, wq)):
            nc.sync.dma_start(out=w_sb[:], in_=w.rearrange("(t p) m -> p t m", p=P))
        wo_sb = wts.tile([P, KT, DIM], PDT, tag="wo")

        qT = {}
        kT = {}
        v_sb = {}

        # out-proj emission lags the a2a launch by one chunk (carried across
        # reps) so its psum slots / collective waits never sit in front of
        # later attention work in the in-order engine streams
        pending = []

        def _emit_outproj(tag, ci, a2a_sb):
            for rt in range(CR // P):        # 128-row tiles of the chunk
                out_sb = osp.tile([P, DIM], FP32, tag="outsb",
                                  name=f"osb{tag}_{rt}")
                for ns in range(2):
                    po = ps.tile([P, QW], FP32, tag="po",
                                 name=f"po{tag}_{rt}{ns}")
                    for j in range(KT):
                        nc.tensor.matmul(
                            po[:], a2a_sb[:, j, rt * P:(rt + 1) * P],
                            wo_sb[:, j, ns * QW:(ns + 1) * QW],
                            start=(j == 0), stop=(j == KT - 1),
                        )
                    nc.vector.tensor_scalar_mul(
                        out_sb[:, ns * QW:(ns + 1) * QW], po[:],
                        1.0 / A2A_S)
                row0 = ci * CR + rt * P
                nc.sync.dma_start(out=out[row0:row0 + P, :],
                                  in_=out_sb[:])

        for rep in range(reps):
            # prefetch x for both batches (DMA queues run ahead)
            xts_all = {}
            for b in range(B):
                xts_all[b] = []
                for kt in range(KT):
                    xtile = xtp.tile([P, N], BF16, tag="xt", name=f"xt{rep}_{b}_{kt}")
                    nc.sync.dma_start(out=xtile[:], in_=xt[b, kt * P:(kt + 1) * P, :])
                    xts_all[b].append(xtile)

            if rep == 0:
                # wo is first needed by the out-proj (~100us in); keep its
                # 2MB DMA out of the critical head of the pipeline
                nc.sync.dma_start(out=wo_sb[:],
                                  in_=wo.rearrange("(t p) m -> p t m", p=P))

            for b in range(B):
                # --------- phase 1a: projections for this batch -------------
                xts = xts_all[b]
                qT[b] = qkp.tile([P, N], MDT, tag="qT", name=f"qT{rep}_{b}")
                kT[b] = qkp.tile([P, N], MDT, tag="kT", name=f"kT{rep}_{b}")
                vT = vtp.tile([P, N], BF16, tag="vT", name=f"vT{rep}_{b}")

                # k first, then v (attention needs all keys/values), then q
                # (query blocks feed attention blocks progressively)
                for w_sb, dst in ((wk_sb, kT[b]), (wv_sb, vT), (wq_sb, qT[b])):
                    for jq in range(NJH):
                        qs = slice(jq * QW, (jq + 1) * QW)
                        pj = ps.tile([P, QW], FP32, tag="fill", bufs=1,
                                     name=f"pj{rep}_{b}{jq}")
                        for kt in range(KT):
                            nc.tensor.matmul(
                                pj[:], w_sb[:, kt, :], xts[kt][:, qs],
                                start=(kt == 0), stop=(kt == KT - 1),
                            )
                        nc.vector.tensor_copy(dst[:, qs], pj[:])

                # transpose vT -> v [n, 130] via PE
                # layout per key tile: [vA(64) | ones | vB(64) | ones]; the
                # ones column makes PV also accumulate the softmax
                # denominator (psum row 64; scaled 1/A2A_S)
                v_sb[b] = vsp.tile([P, NKT, 130], PDT, tag="vsb",
                                   name=f"vsb{rep}_{b}")
                nc.vector.tensor_copy(v_sb[b][:, :, 64], invs_f32[:, 0:NKT])
                nc.vector.tensor_copy(v_sb[b][:, :, 129], invs_f32[:, 0:NKT])
                for t in range(NKT):
                    tp = ps.tile([P, P], BF16, tag="fill", bufs=1,
                                 name=f"tp{rep}_{b}{t}")
                    nc.tensor.transpose(tp[:], vT[:, t * P:(t + 1) * P],
                                        ident_bf[:])
                    nc.vector.tensor_copy(v_sb[b][:, t, 0:64], tp[:, 0:64])
                    nc.vector.tensor_copy(v_sb[b][:, t, 65:129], tp[:, 64:128])

                # --------- phase 1b: attention + chunked a2a ----------------
                for h2 in range(CPB):
                    attTc = atc.tile([P, NSUB, QW], ADT, tag="attc",
                                     name=f"attc{rep}_{b}{h2}")
                    for sub in range(NSUB):
                        jh = NSUB * h2 + sub
                        qs = slice(jh * QW, (jh + 1) * QW)
                        pvA = ps.tile([65, QW], FP32, tag="pvA",
                                      name=f"pvA{rep}_{b}{jh}")
                        pvB = ps.tile([65, QW], FP32, tag="pvB",
                                      name=f"pvB{rep}_{b}{jh}")
                        for t in range(NKT):
                            ks = slice(t * P, (t + 1) * P)
                            # both heads' scores side by side in one 2-bank
                            # psum tile so a single exp covers them
                            sAB = ps.tile([P, 2, QW], FP32, tag="sAB", bufs=2,
                                          name=f"sAB{rep}_{b}{jh}{t}")
                            nc.tensor.matmul(sAB[:, 0, :], kT[b][0:64, ks],
                                             qT[b][0:64, qs])
                            if not ABLATE_SB:
                                nc.tensor.matmul(sAB[:, 1, :],
                                                 kT[b][64:128, ks],
                                                 qT[b][64:128, qs])
                            pAB = pp.tile([P, 2, QW], PDT, tag="pAB",
                                          name=f"pAB{rep}_{b}{jh}{t}")
                            nc.scalar.activation(pAB[:], sAB[:],
                                                 mybir.ActivationFunctionType.Exp,
                                                 bias=mb_sb[:, b, t:t + 1],
                                                 scale=SCALE)
                            st, sp = (t == 0), (t == NKT - 1)
                            nc.tensor.matmul(pvA[:], v_sb[b][:, t, 0:65],
                                             pAB[:, 0, :], start=st, stop=sp)
                            if not ABLATE_PV:
                                nc.tensor.matmul(pvB[:], v_sb[b][:, t, 65:130],
                                                 pAB[:, 1, :], start=st,
                                                 stop=sp)
                        # normalize: att = pv[0:64] * (1/pv[64]): reciprocal,
                        # K=1 ones-matmul broadcast into psum (po tag: its
                        # out-proj users lag a chunk, so no coupling into the
                        # attention critical path), then DVE multiply
                        for h, pv in enumerate((pvA, pvB)):
                            rc = sml.tile([1, QW], MDT, tag="rc",
                                          name=f"rc{rep}_{b}{jh}{h}")
                            nc.vector.reciprocal(rc[:], pv[64:65, :])
                            bc = ps.tile([64, QW], FP32, tag="po",
                                         name=f"bc{rep}_{b}{jh}{h}")
                            nc.tensor.matmul(bc[:], ones_col[:], rc[:])
                            bc_sb = sml.tile([64, QW], FP32, tag="bcs",
                                             name=f"bcs{rep}_{b}{jh}{h}")
                            nc.vector.tensor_copy(bc_sb[:], bc[:])
                            nc.vector.tensor_mul(attTc[64 * h:64 * (h + 1), sub, :],
                                                 pv[0:64, :], bc_sb[:])

                    # ---- chunk (b, h2): launch a2a now; out-proj deferred ---
                    ci = CPB * b + h2
                    a2a_in = dram.tile([NCORES * P, CR], ADT, tag="a2i",
                                       name=f"a2i{rep}_{ci}")
                    nc.sync.dma_start(
                        out=a2a_in.rearrange("(s p) f -> p s f", p=P),
                        in_=attTc[:].rearrange("p a (s f) -> p (a s) f", f=CR))
                    a2a_out = dram.tile([NCORES * P, CR], ADT, tag="a2o",
                                        name=f"a2o{rep}_{ci}")
                    if not ABLATE_COLL:
                        nc.gpsimd.collective_compute(
                            "AllToAll", mybir.AluOpType.bypass,
                            replica_groups=[list(range(NCORES))],
                            ins=[a2a_in.opt()], outs=[a2a_out.opt()],
                        )
                    a2a_sb = a2s.tile([P, NCORES, CR], ADT, tag="a2s",
                                      name=f"a2s{rep}_{ci}")
                    nc.sync.dma_start(
                        out=a2a_sb[:],
                        in_=a2a_out.rearrange("(s p) f -> p s f", p=P))
                    pending.append((f"{rep}_{ci}", ci, a2a_sb))
                    if len(pending) > 1:
                        _emit_outproj(*pending.pop(0))
        while pending:
            _emit_outproj(*pending.pop(0))

    nc.finalize()
    return nc


_CACHED = {}


def _get_program(reps=1):
    key = (MM_MODE, reps, ABLATE_SB, ABLATE_PV, ABLATE_COLL)
    if key not in _CACHED:
        _CACHED[key] = build_program(reps)
    return _CACHED[key]


def make_in_maps(x, mask, Wq, Wkv, Wo):
    """Host-side shard prep: per-core input dicts."""
    x = np.asarray(x, dtype=np.float32)
    mask = np.asarray(mask)
    Wq = np.asarray(Wq, dtype=np.float32)
    Wkv = np.asarray(Wkv, dtype=np.float32)
    Wo = np.asarray(Wo, dtype=np.float32)

    bf16 = __import__("ml_dtypes").bfloat16
    xT = np.ascontiguousarray(x.transpose(0, 2, 1)).astype(bf16)  # [B, DIM, N]
    mbias = np.where(mask, 0.0, MASK_NEG).astype(np.float32)   # [B, N]
    mbias = np.ascontiguousarray(
        mbias.reshape(B, NKT, P).transpose(0, 2, 1))           # [B, 128, NKT]

    in_maps = []
    for c in range(NCORES):
        cs = slice(c * CW, (c + 1) * CW)
        in_maps.append({
            "xt": xT,
            "wq": np.ascontiguousarray(Wq[:, cs]).astype(bf16),
            "wk": np.ascontiguousarray(Wkv[:, cs]).astype(bf16),
            "wv": np.ascontiguousarray(Wkv[:, DI + c * CW: DI + (c + 1) * CW]).astype(bf16),
            "wo": Wo.astype(bf16),
            "mb": mbias,
        })
    return in_maps


def assemble(results):
    # out rows per core are in chunk order: [(b, h2), CR] with the core's
    # rows at global n = h2*(N//CPB) + c*CR + r
    outs = np.stack([np.asarray(results[c]["out"]) for c in range(NCORES)])
    outs = outs.reshape(NCORES, B, CPB, CR, DIM)    # [c, b, h2, r, d]
    outs = outs.transpose(1, 2, 0, 3, 4)            # [b, h2, c, r, d]
    return np.ascontiguousarray(outs.reshape(B, N, DIM)).astype(np.float32)


def kernel(x, mask, Wq, Wkv, Wo):
    from concourse.bass_utils import run_bass_kernel_spmd

    nc = _get_program()
    in_maps = make_in_maps(x, mask, Wq, Wkv, Wo)
    res = run_bass_kernel_spmd(nc, in_maps, list(range(NCORES)))
    return assemble(res.results)
